# revision 13
# baseline (speedup 1.0000x reference)
"""Trainium2 Bass kernel for the MoE-routed 3-layer LoRA MLP.

Strategy: pure data-parallel over the batch (16384 rows -> 2048 per core,
8 cores, no collectives). On-device layout is feature-major (transposed):
activations live as [features, batch] so every matmul contracts over the
partition dimension without any on-device transposes. All matmul operands
are bf16 (PSUM accumulation is f32); the tiny domain-routing network runs
in f32 on device and is folded into a per-domain gamma = zeta * alpha
table, gathered to per-token scale rows via a one-hot matmul.

Per core the three layers are fused column-by-column (4 columns of 512
tokens): weights for all layers stay SBUF-resident; h1/h2 never touch DRAM.
"""

import json

import numpy as np
import ml_dtypes

import concourse.bass as bass
import concourse.tile as tile
from concourse import mybir
from concourse.bass_utils import run_bass_kernel_spmd

F32 = mybir.dt.float32
BF16 = mybir.dt.bfloat16
AF = mybir.ActivationFunctionType
ALU = mybir.AluOpType
AX = mybir.AxisListType

N_CORES = 8
BSZ, D0, D1, D2, D3 = 16384, 2048, 2048, 1024, 512
E, RK, M, H, L = 4, 8, 8, 64, 3
B_LOC = BSZ // N_CORES  # 2048
NT = 4                  # batch columns per core
NB = B_LOC // NT        # 512
BF_NP = ml_dtypes.bfloat16


# ---------------------------------------------------------------------------
# BIR post-pass: this container's walrus rejects instructions carrying more
# than one semaphore wait; split extras onto preceding same-engine NoOps
# (the engine sequencer processes waits before the instruction, so this is
# semantics-preserving).
# ---------------------------------------------------------------------------
def _split_waits(bir, max_waits=1):
    counter = [0]

    def fix_block(bb):
        new_instructions = []
        for ins in bb.get("instructions", []):
            si = ins.get("sync_info") or {}
            waits = si.get("on_wait") or []
            if len(waits) > max_waits:
                head, tail = waits[:-max_waits], waits[-max_waits:]
                for i in range(0, len(head), max_waits):
                    counter[0] += 1
                    new_instructions.append(
                        {
                            "engine": ins["engine"],
                            "ins": [],
                            "name": f"I-waitsplit-{counter[0]}",
                            "opcode": "Drain",
                            "outs": [],
                            "sync_info": {
                                "on_update": [],
                                "on_wait": head[i : i + max_waits],
                            },
                        }
                    )
                si = dict(si)
                si["on_wait"] = tail
                ins = dict(ins)
                ins["sync_info"] = si
            new_instructions.append(ins)
        if "instructions" in bb:
            bb["instructions"] = new_instructions
        for inner in bb.get("blocks", []):
            fix_block(inner)

    for fn in bir.get("functions", []):
        for bb in fn.get("blocks", []):
            fix_block(bb)
    return bir


def _patch_bass_json(nc):
    orig = nc.to_json_bytes

    def wrapped(*a, **k):
        return json.dumps(_split_waits(json.loads(orig(*a, **k)))).encode()

    nc.to_json_bytes = wrapped


# ---------------------------------------------------------------------------
# Routing: compute gexp [8, 96] f32 where
#   gexp[m, l*32 + e*8 + r] = zeta_agg[m, l] * alpha_agg[m, l, e]
# ---------------------------------------------------------------------------
def _build_routing(nc, const, small, psum, dram, warmup_fn=None):
    ML = M * L
    rin = nc.dram_tensor("rin", [2 * H, ML], F32, kind="ExternalInput")
    wi1t = nc.dram_tensor("wi1t", [2 * H, H], F32, kind="ExternalInput")
    wa1t = nc.dram_tensor("wa1t", [2 * H, H], F32, kind="ExternalInput")
    bi1v = nc.dram_tensor("bi1v", [H], F32, kind="ExternalInput")
    ba1v = nc.dram_tensor("ba1v", [H], F32, kind="ExternalInput")
    wi2b = nc.dram_tensor("wi2b", [H + 1, 1], F32, kind="ExternalInput")
    wa2b = nc.dram_tensor("wa2b", [H + 1, E], F32, kind="ExternalInput")
    gatet = nc.dram_tensor("gatet", [M, M], F32, kind="ExternalInput")
    rbt = nc.dram_tensor("rbt", [M, M], F32, kind="ExternalInput")

    rin_s = const.tile([2 * H, ML], F32, tag="rin")
    wi1t_s = const.tile([2 * H, H], F32, tag="wi1t")
    wa1t_s = const.tile([2 * H, H], F32, tag="wa1t")
    bi1_s = const.tile([H, 1], F32, tag="bi1")
    ba1_s = const.tile([H, 1], F32, tag="ba1")
    wi2b_s = const.tile([H + 1, 1], F32, tag="wi2b")
    wa2b_s = const.tile([H + 1, E], F32, tag="wa2b")
    gatet_s = const.tile([M, M], F32, tag="gatet")
    rbt_s = const.tile([M, M], F32, tag="rbt")
    for t, d in [
        (rin_s, rin), (wi1t_s, wi1t), (wa1t_s, wa1t),
        (wi2b_s, wi2b), (wa2b_s, wa2b), (gatet_s, gatet), (rbt_s, rbt),
    ]:
        nc.sync.dma_start(out=t[:], in_=d[:])
    nc.sync.dma_start(out=bi1_s[:], in_=bi1v.rearrange("(h one) -> h one", one=1))
    nc.sync.dma_start(out=ba1_s[:], in_=ba1v.rearrange("(h one) -> h one", one=1))

    # router hidden layers, with an extra ones-row to fold the output bias
    hz_ext = small.tile([H + 1, ML], F32, tag="hz")
    ha_ext = small.tile([H + 1, ML], F32, tag="ha")
    for wt, bt, ext in [(wi1t_s, bi1_s, hz_ext), (wa1t_s, ba1_s, ha_ext)]:
        ps = psum.tile([H, ML], F32, tag="rpsum")
        nc.tensor.matmul(ps[:], wt[:], rin_s[:], start=True, stop=True)
        nc.scalar.activation(ext[0:H, :], ps[:], AF.Relu, bias=bt[:])
        nc.vector.memset(ext[H : H + 1, :], 1.0)

    # zeta logits [24,1] -> [8,3] via DRAM bounce
    zps = psum.tile([ML, 1], F32, tag="rpsum")
    nc.tensor.matmul(zps[:], hz_ext[:], wi2b_s[:], start=True, stop=True)
    z24 = small.tile([ML, 1], F32, tag="z24")
    nc.vector.tensor_copy(z24[:], zps[:])
    zdram = dram.tile([ML, 1], F32, tag="zdram")
    nc.sync.dma_start(out=zdram[:], in_=z24[:])
    zl = small.tile([M, L], F32, tag="zl")
    nc.sync.dma_start(out=zl[:], in_=zdram.rearrange("(m l) one -> m (l one)", m=M))

    # alpha logits [24,4]
    aps = psum.tile([ML, E], F32, tag="rpsum")
    nc.tensor.matmul(aps[:], ha_ext[:], wa2b_s[:], start=True, stop=True)
    al = small.tile([ML, E], F32, tag="al")
    nc.vector.tensor_copy(al[:], aps[:])
    if warmup_fn is not None:
        warmup_fn()

    # zeta sparse softmax over L=3, keep top-2 (drop the min)
    zneg = small.tile([M, L], F32, tag="zneg")
    nc.vector.tensor_scalar_mul(zneg[:], zl[:], -1.0)
    zmin = small.tile([M, 1], F32, tag="zmin")
    nc.vector.reduce_max(zmin[:], zneg[:], axis=AX.X)
    nc.vector.tensor_scalar_mul(zmin[:], zmin[:], -1.0)
    zmax = small.tile([M, 1], F32, tag="zmax")
    nc.vector.reduce_max(zmax[:], zl[:], axis=AX.X)
    zmaxn = small.tile([M, 1], F32, tag="zmaxn")
    nc.vector.tensor_scalar_mul(zmaxn[:], zmax[:], -1.0)
    ze = small.tile([M, L], F32, tag="ze")
    nc.scalar.activation(ze[:], zl[:], AF.Exp, bias=zmaxn[:])
    zmask = small.tile([M, L], F32, tag="zmask")
    nc.vector.tensor_scalar(zmask[:], zl[:], zmin[:], None, ALU.is_gt)
    nc.vector.tensor_mul(ze[:], ze[:], zmask[:])
    zs = small.tile([M, 1], F32, tag="zs")
    nc.vector.reduce_sum(zs[:], ze[:], axis=AX.X)
    zrs = small.tile([M, 1], F32, tag="zrs")
    nc.vector.reciprocal(zrs[:], zs[:])
    zeta_all = small.tile([M, L], F32, tag="zeta_all")
    nc.vector.tensor_scalar_mul(zeta_all[:], ze[:], zrs[:])

    # alpha sparse softmax over E=4, keep top-2 (threshold = 2nd max)
    m1 = small.tile([ML, 1], F32, tag="m1")
    nc.vector.reduce_max(m1[:], al[:], axis=AX.X)
    m1n = small.tile([ML, 1], F32, tag="m1n")
    nc.vector.tensor_scalar_mul(m1n[:], m1[:], -1.0)
    meq = small.tile([ML, E], F32, tag="meq")
    nc.vector.tensor_scalar(meq[:], al[:], m1[:], None, ALU.is_equal)
    nc.vector.tensor_scalar_mul(meq[:], meq[:], 1e30)
    v2 = small.tile([ML, E], F32, tag="v2")
    nc.vector.tensor_sub(v2[:], al[:], meq[:])
    m2 = small.tile([ML, 1], F32, tag="m2")
    nc.vector.reduce_max(m2[:], v2[:], axis=AX.X)
    keep = small.tile([ML, E], F32, tag="keep")
    nc.vector.tensor_scalar(keep[:], al[:], m2[:], None, ALU.is_ge)
    ae = small.tile([ML, E], F32, tag="ae")
    nc.scalar.activation(ae[:], al[:], AF.Exp, bias=m1n[:])
    nc.vector.tensor_mul(ae[:], ae[:], keep[:])
    as_ = small.tile([ML, 1], F32, tag="as_")
    nc.vector.reduce_sum(as_[:], ae[:], axis=AX.X)
    ars = small.tile([ML, 1], F32, tag="ars")
    nc.vector.reciprocal(ars[:], as_[:])
    alpha_all = small.tile([ML, E], F32, tag="alpha_all")
    nc.vector.tensor_scalar_mul(alpha_all[:], ae[:], ars[:])

    # [24,4] -> [8,12] via DRAM bounce
    adram = dram.tile([ML, E], F32, tag="adram")
    nc.sync.dma_start(out=adram[:], in_=alpha_all[:])
    alpha8 = small.tile([M, L * E], F32, tag="alpha8")
    nc.sync.dma_start(out=alpha8[:], in_=adram.rearrange("(m l) e -> m (l e)", m=M))

    # RuT[n,m] = softplus(gate[m,n]) * Rb[m,n]   (softplus = ln(1+exp))
    rut = small.tile([M, M], F32, tag="rut")
    nc.scalar.activation(rut[:], gatet_s[:], AF.Exp)
    nc.vector.tensor_scalar_add(rut[:], rut[:], 1.0)
    nc.scalar.activation(rut[:], rut[:], AF.Ln)
    nc.vector.tensor_mul(rut[:], rut[:], rbt_s[:])

    # aggregate [zeta(3) | alpha(12) | ones(1)] through RuT, then normalize
    W16 = L + L * E + 1
    agg_rhs = small.tile([M, W16], F32, tag="agg_rhs")
    nc.vector.tensor_copy(agg_rhs[:, 0:L], zeta_all[:])
    nc.vector.tensor_copy(agg_rhs[:, L : L + L * E], alpha8[:])
    nc.vector.memset(agg_rhs[:, W16 - 1 : W16], 1.0)
    agg_ps = psum.tile([M, W16], F32, tag="rpsum")
    nc.tensor.matmul(agg_ps[:], rut[:], agg_rhs[:], start=True, stop=True)
    rsum = small.tile([M, 1], F32, tag="rsum")
    nc.vector.tensor_scalar_max(rsum[:], agg_ps[:, W16 - 1 : W16], 1e-12)
    rrs = small.tile([M, 1], F32, tag="rrs")
    nc.vector.reciprocal(rrs[:], rsum[:])
    table = small.tile([M, L + L * E], F32, tag="table")
    nc.vector.tensor_scalar_mul(table[:], agg_ps[:, 0 : L + L * E], rrs[:])

    # gamma12[m, l*4+e] = zeta[m,l] * alpha[m, l*4+e]
    zexp = small.tile([M, L * E], F32, tag="zexp")
    zview = zexp.rearrange("p (l e) -> p l e", e=E)
    for e in range(E):
        nc.vector.tensor_copy(zview[:, :, e], table[:, 0:L])
    gamma12 = small.tile([M, L * E], F32, tag="gamma12")
    nc.vector.tensor_mul(gamma12[:], table[:, L : L + L * E], zexp[:])

    # expand over rank r: gexp[:, l*32 + e*8 + r] = gamma12[:, l*4+e]
    gexp = small.tile([M, L * E * RK], F32, tag="gexp")
    gview = gexp.rearrange("p (le r) -> p le r", r=RK)
    for r in range(RK):
        nc.vector.tensor_copy(gview[:, :, r], gamma12[:])
    return gexp


# ---------------------------------------------------------------------------
# Full per-core graph
# ---------------------------------------------------------------------------
def _build(nc):
    DIMS = [(D0, D1), (D1, D2), (D2, D3)]

    xt = nc.dram_tensor("xt", [D0, B_LOC], BF16, kind="ExternalInput")
    onehot = nc.dram_tensor("onehot", [M, B_LOC], F32, kind="ExternalInput")
    combine_d = nc.dram_tensor("combine", [128, 128], BF16, kind="ExternalInput")
    wts = [
        nc.dram_tensor(f"w{l + 1}t", [i, o], BF16, kind="ExternalInput")
        for l, (i, o) in enumerate(DIMS)
    ]
    ats = [
        nc.dram_tensor(f"a{l + 1}t", [i, E * RK], BF16, kind="ExternalInput")
        for l, (i, _) in enumerate(DIMS)
    ]
    lbs = [
        nc.dram_tensor(f"lb{l + 1}", [128, o], BF16, kind="ExternalInput")
        for l, (_, o) in enumerate(DIMS)
    ]
    biases = [
        nc.dram_tensor(f"bias{l + 1}", [o], F32, kind="ExternalInput")
        for l, (_, o) in enumerate(DIMS)
    ]
    out_d = nc.dram_tensor("out", [D3, B_LOC], F32, kind="ExternalOutput")

    with tile.TileContext(nc) as tc:
        with (
            tc.tile_pool(name="const", bufs=1) as const,
            tc.tile_pool(name="small", bufs=1) as small,
            tc.tile_pool(name="rpsum", bufs=1, space="PSUM") as rpsum,
            tc.tile_pool(name="dram", bufs=1, space="DRAM") as dram,
            tc.tile_pool(name="wpool", bufs=1) as wpool,
            tc.tile_pool(name="gpool", bufs=1) as gpool,
            tc.tile_pool(name="onp", bufs=4) as onp,
            tc.tile_pool(name="xcol", bufs=16) as xcolp,
            tc.tile_pool(name="h1", bufs=16) as h1p,
            tc.tile_pool(name="h2", bufs=10) as h2p,
            tc.tile_pool(name="oc", bufs=3) as ocp,
            tc.tile_pool(name="tw", bufs=4) as twp,
            tc.tile_pool(name="mmps", bufs=4, space="PSUM") as mmps,
            tc.tile_pool(name="warmp", bufs=1, space="PSUM") as warmp,
            tc.tile_pool(name="tps", bufs=2, space="PSUM") as tps,
        ):
            # --- PE warmup: keep HAM hot while DMAs stream ------------------
            warm_src = small.tile([128, 128], BF16, tag="warm_src")
            nc.vector.memset(warm_src[:], 0.0)
            warm_sink = dram.tile([128, 128], BF16, tag="warm_sink")
            warm_ps = warmp.tile([128, 128], F32, tag="warm", name="warm_ps")

            def warmup(count, label):
                for i in range(count):
                    nc.tensor.matmul(warm_ps[:], warm_src[:], warm_src[:],
                                     start=True, stop=True)

            # one-hot slices early so their DMAs precede the weight bulk
            on_tiles = []
            for n in range(NT):
                on_t = onp.tile([M, NB], F32, tag="on", name=f"on{n}")
                nc.sync.dma_start(out=on_t[:], in_=onehot[:, n * NB : (n + 1) * NB])
                on_tiles.append(on_t)
            combine_t = const.tile([128, 128], BF16, tag="combine")
            nc.sync.dma_start(out=combine_t[:], in_=combine_d[:])

            warmup(24, "a")
            gexp = _build_routing(nc, const, small, rpsum, dram,
                                  warmup_fn=lambda: warmup(120, "b"))

            # resident weights / inputs: layer-1 + first column first
            w_tiles = [[] for _ in range(L)]
            a_tiles = [[] for _ in range(L)]
            lb_tiles = [None] * L
            b_tiles = [None] * L

            def load_layer_small(l):
                IN, OUT = DIMS[l]
                lb_tiles[l] = wpool.tile([128, OUT], BF16, tag=f"lb{l}", name=f"lb{l}")
                nc.sync.dma_start(out=lb_tiles[l][:], in_=lbs[l][:])
                b_tiles[l] = wpool.tile([128, OUT // 128], F32, tag=f"b{l}", name=f"b{l}")
                nc.sync.dma_start(
                    out=b_tiles[l][:], in_=biases[l].rearrange("(o p) -> p o", p=128)
                )
                for k in range(IN // 128):
                    at_t = wpool.tile([128, E * RK], BF16, tag=f"a{l}_{k}", name=f"a{l}_{k}")
                    nc.sync.dma_start(out=at_t[:], in_=ats[l][k * 128 : (k + 1) * 128, :])
                    a_tiles[l].append(at_t)

            def load_layer(l):
                IN, OUT = DIMS[l]
                nchunk = 4 if l == 0 else 1
                cw = OUT // nchunk
                for k in range(IN // 128):
                    wt_t = wpool.tile([128, OUT], BF16, tag=f"w{l}_{k}", name=f"w{l}_{k}")
                    w_tiles[l].append(wt_t)
                for c in range(nchunk):
                    for k in range(IN // 128):
                        nc.sync.dma_start(
                            out=w_tiles[l][k][:, c * cw : (c + 1) * cw],
                            in_=wts[l][k * 128 : (k + 1) * 128, c * cw : (c + 1) * cw],
                        )

            def load_xcol(n):
                cols = []
                for k in range(D0 // 128):
                    xk = xcolp.tile([128, NB], BF16, tag="xcol", name=f"x{n}_{k}")
                    nc.sync.dma_start(
                        out=xk[:], in_=xt[k * 128 : (k + 1) * 128, n * NB : (n + 1) * NB]
                    )
                    cols.append(xk)
                return cols

            gammas = [
                gpool.tile([128, B_LOC], BF16, tag=f"g{l}", name=f"gamma{l}")
                for l in range(L)
            ]

            def emit_gather():
                for l in range(L):
                    g4 = small.tile([M, 128], F32, tag=f"gexp4_{l}", name=f"gexp4_{l}")
                    for g in range(4):
                        nc.vector.tensor_copy(
                            g4[:, g * 32 : (g + 1) * 32], gexp[:, l * 32 : (l + 1) * 32]
                        )
                    for n in range(NT):
                        gps = tps.tile([128, NB], F32, tag="tpsum")
                        nc.tensor.matmul(gps[:], g4[:], on_tiles[n][:],
                                         start=True, stop=True)
                        nc.vector.tensor_copy(gammas[l][:, n * NB : (n + 1) * NB], gps[:])

            for l in range(L):
                load_layer_small(l)
            first_cols = load_xcol(0)
            load_layer(0)
            load_layer(1)
            load_layer(2)

            def lora_t4(l, n, cols, KT):
                """LoRA A-side, col-group packed: 4 concurrent partial
                accumulations in one PSUM bank, then one combine matmul that
                also replicates t over the four row groups."""
                part = tps.tile([128, NB], F32, tag="tpsum")
                for k in range(KT):
                    g = k % 4
                    nc.tensor.matmul(
                        part[g * 32 : (g + 1) * 32, :], a_tiles[l][k][:], cols[k][:],
                        start=(k < 4), stop=(k >= KT - 4), tile_position=(0, g * 32),
                    )
                pt = twp.tile([128, NB], BF16, tag="tw", name=f"pt{l}_{n}")
                nc.vector.tensor_copy(pt[:], part[:])
                t4 = tps.tile([128, NB], F32, tag="tpsum")
                nc.tensor.matmul(t4[:], combine_t[:], pt[:], start=True, stop=True)
                return t4

            # main fused pipeline: per batch-column, all three layers
            for n in range(NT):
                cols = first_cols if n == 0 else load_xcol(n)
                for l, (IN, OUT) in enumerate(DIMS):
                    KT, OT = IN // 128, OUT // 128
                    first = n == 0 and l == 0
                    t4 = lora_t4(l, n, cols, KT)
                    tw = None
                    if not first:
                        tw = twp.tile([128, NB], BF16, tag="tw")
                        nc.vector.tensor_mul(
                            tw[:], t4[:], gammas[l][:, n * NB : (n + 1) * NB]
                        )

                    nxt = []
                    ogroups = list(range(0, OT, 4))
                    for og in ogroups:
                        gw = min(4, OT - og)
                        pss = []
                        for i in range(gw):
                            o = og + i
                            ps = mmps.tile([128, NB], F32, tag="mm")
                            for k in range(KT):
                                nc.tensor.matmul(
                                    ps[:], w_tiles[l][k][:, o * 128 : (o + 1) * 128],
                                    cols[k][:], start=(k == 0), stop=False,
                                )
                            pss.append(ps)
                        if first and og == 0:
                            # routing-dependent gather lands here, overlapped
                            # by the first group of W-matmuls
                            emit_gather()
                            tw = twp.tile([128, NB], BF16, tag="tw", name="tw_first")
                            nc.vector.tensor_mul(tw[:], t4[:], gammas[0][:, 0:NB])
                        for i in range(gw):
                            o = og + i
                            nc.tensor.matmul(
                                pss[i][:],
                                lb_tiles[l][i * 32 : (i + 1) * 32, o * 128 : (o + 1) * 128],
                                tw[i * 32 : (i + 1) * 32, :],
                                start=False, stop=True, tile_position=(i * 32, 0),
                            )
                        for i in range(gw):
                            o = og + i
                            if l < 2:
                                pool = h1p if l == 0 else h2p
                                ot = pool.tile([128, NB], BF16, tag=f"h{l + 1}", name=f"h{l}_{n}_{o}")
                                nc.scalar.activation(
                                    ot[:], pss[i][:], AF.Relu, bias=b_tiles[l][:, o : o + 1]
                                )
                                nxt.append(ot)
                            else:
                                ot = ocp.tile([128, NB], F32, tag="oc", name=f"oc{n}_{o}")
                                nc.scalar.activation(
                                    ot[:], pss[i][:], AF.Relu, bias=b_tiles[l][:, o : o + 1]
                                )
                                nc.sync.dma_start(
                                    out=out_d[o * 128 : (o + 1) * 128, n * NB : (n + 1) * NB],
                                    in_=ot[:],
                                )
                    cols = nxt
            wout = small.tile([128, 128], BF16, tag="warm_out", name="warmout")
            nc.vector.tensor_copy(wout[:], warm_ps[:])
            nc.sync.dma_start(out=warm_sink[:], in_=wout[:])
    return nc


_CACHED = {}


def _get_nc():
    if "nc" not in _CACHED:
        nc = bass.Bass()
        _build(nc)
        _patch_bass_json(nc)
        _CACHED["nc"] = nc
    return _CACHED["nc"]


def kernel(**inputs) -> np.ndarray:
    x = np.asarray(inputs["x"], np.float32)
    ids = np.asarray(inputs["domain_ids"]).astype(np.int64)
    f32 = lambda a: np.ascontiguousarray(np.asarray(a), np.float32)
    bf = lambda a: np.ascontiguousarray(np.asarray(a, np.float32).astype(BF_NP))

    W = [f32(inputs[f"W{i}"]) for i in (1, 2, 3)]
    Bv = [f32(inputs[f"b{i}"]) for i in (1, 2, 3)]
    A = [f32(inputs[f"A{i}"]) for i in (1, 2, 3)]
    Bl = [f32(inputs[f"B{i}"]) for i in (1, 2, 3)]

    dom_emb, layer_pos = f32(inputs["dom_emb"]), f32(inputs["layer_pos"])
    rin = np.concatenate(
        [
            np.broadcast_to(dom_emb[:, None, :], (M, L, H)),
            np.broadcast_to(layer_pos[None, :, :], (M, L, H)),
        ],
        axis=-1,
    ).reshape(M * L, 2 * H).T

    shared = {
        "wi1t": f32(inputs["Wi1"]).T, "wa1t": f32(inputs["Wa1"]).T,
        "bi1v": f32(inputs["bi1"]), "ba1v": f32(inputs["ba1"]),
        "wi2b": np.concatenate([f32(inputs["Wi2"]).T, f32(inputs["bi2"])[None, :]], 0),
        "wa2b": np.concatenate([f32(inputs["Wa2"]).T, f32(inputs["ba2"])[None, :]], 0),
        "gatet": f32(inputs["gate_logits"]).T, "rbt": f32(inputs["R_benefit"]).T,
        "rin": rin,
    }
    shared = {k: f32(v) for k, v in shared.items()}
    shared["combine"] = bf(np.tile(np.eye(E * RK, dtype=np.float32), (4, 4)))
    for l in range(3):
        shared[f"w{l + 1}t"] = bf(W[l].T)
        shared[f"a{l + 1}t"] = bf(A[l].reshape(E * RK, -1).T)
        shared[f"lb{l + 1}"] = bf(np.tile(Bl[l].transpose(0, 2, 1).reshape(E * RK, -1), (4, 1)))
        shared[f"bias{l + 1}"] = Bv[l]

    in_maps = []
    for i in range(N_CORES):
        sl = slice(i * B_LOC, (i + 1) * B_LOC)
        m = dict(shared)
        m["xt"] = bf(x[sl].T)
        m["onehot"] = np.ascontiguousarray(
            (ids[sl][None, :] == np.arange(M)[:, None]).astype(np.float32)
        )
        in_maps.append(m)

    nc = _get_nc()
    res = run_bass_kernel_spmd(nc, in_maps, core_ids=list(range(N_CORES)))
    return np.concatenate(
        [np.asarray(res.results[i]["out"], np.float32).T for i in range(N_CORES)], axis=0
    )


# revision 14
# speedup vs baseline: 1.0397x; 1.0397x over previous
"""Trainium2 Bass kernel for the MoE-routed 3-layer LoRA MLP.

Strategy: pure data-parallel over the batch (16384 rows -> 2048 per core,
8 cores, no collectives). On-device layout is feature-major (transposed):
activations live as [features, batch] so every matmul contracts over the
partition dimension without any on-device transposes. All matmul operands
are bf16 (PSUM accumulation is f32); the tiny domain-routing network runs
in f32 on device and is folded into a per-domain gamma = zeta * alpha
table, gathered to per-token scale rows via a one-hot matmul.

Per core the three layers are fused column-by-column (4 columns of 512
tokens): weights for all layers stay SBUF-resident; h1/h2 never touch DRAM.
"""

import json

import numpy as np
import ml_dtypes

import concourse.bass as bass
import concourse.tile as tile
from concourse import mybir
from concourse.bass_utils import run_bass_kernel_spmd

F32 = mybir.dt.float32
BF16 = mybir.dt.bfloat16
AF = mybir.ActivationFunctionType
ALU = mybir.AluOpType
AX = mybir.AxisListType

N_CORES = 8
BSZ, D0, D1, D2, D3 = 16384, 2048, 2048, 1024, 512
E, RK, M, H, L = 4, 8, 8, 64, 3
B_LOC = BSZ // N_CORES  # 2048
NT = 4                  # batch columns per core
NB = B_LOC // NT        # 512
BF_NP = ml_dtypes.bfloat16


# ---------------------------------------------------------------------------
# BIR post-pass: this container's walrus rejects instructions carrying more
# than one semaphore wait; split extras onto preceding same-engine NoOps
# (the engine sequencer processes waits before the instruction, so this is
# semantics-preserving).
# ---------------------------------------------------------------------------
def _split_waits(bir, max_waits=1):
    counter = [0]

    def fix_block(bb):
        new_instructions = []
        for ins in bb.get("instructions", []):
            si = ins.get("sync_info") or {}
            waits = si.get("on_wait") or []
            if len(waits) > max_waits:
                head, tail = waits[:-max_waits], waits[-max_waits:]
                for i in range(0, len(head), max_waits):
                    counter[0] += 1
                    new_instructions.append(
                        {
                            "engine": ins["engine"],
                            "ins": [],
                            "name": f"I-waitsplit-{counter[0]}",
                            "opcode": "Drain",
                            "outs": [],
                            "sync_info": {
                                "on_update": [],
                                "on_wait": head[i : i + max_waits],
                            },
                        }
                    )
                si = dict(si)
                si["on_wait"] = tail
                ins = dict(ins)
                ins["sync_info"] = si
            new_instructions.append(ins)
        if "instructions" in bb:
            bb["instructions"] = new_instructions
        for inner in bb.get("blocks", []):
            fix_block(inner)

    for fn in bir.get("functions", []):
        for bb in fn.get("blocks", []):
            fix_block(bb)
    return bir


def _patch_bass_json(nc):
    orig = nc.to_json_bytes

    def wrapped(*a, **k):
        return json.dumps(_split_waits(json.loads(orig(*a, **k)))).encode()

    nc.to_json_bytes = wrapped


# ---------------------------------------------------------------------------
# Routing: compute gexp [8, 96] f32 where
#   gexp[m, l*32 + e*8 + r] = zeta_agg[m, l] * alpha_agg[m, l, e]
# ---------------------------------------------------------------------------
def _build_routing(nc, const, small, psum, dram, warmup_fn=None):
    ML = M * L
    rin = nc.dram_tensor("rin", [2 * H, ML], F32, kind="ExternalInput")
    wi1t = nc.dram_tensor("wi1t", [2 * H, H], F32, kind="ExternalInput")
    wa1t = nc.dram_tensor("wa1t", [2 * H, H], F32, kind="ExternalInput")
    bi1v = nc.dram_tensor("bi1v", [H], F32, kind="ExternalInput")
    ba1v = nc.dram_tensor("ba1v", [H], F32, kind="ExternalInput")
    wi2b = nc.dram_tensor("wi2b", [H + 1, 1], F32, kind="ExternalInput")
    wa2b = nc.dram_tensor("wa2b", [H + 1, E], F32, kind="ExternalInput")
    gatet = nc.dram_tensor("gatet", [M, M], F32, kind="ExternalInput")
    rbt = nc.dram_tensor("rbt", [M, M], F32, kind="ExternalInput")

    rin_s = const.tile([2 * H, ML], F32, tag="rin")
    wi1t_s = const.tile([2 * H, H], F32, tag="wi1t")
    wa1t_s = const.tile([2 * H, H], F32, tag="wa1t")
    bi1_s = const.tile([H, 1], F32, tag="bi1")
    ba1_s = const.tile([H, 1], F32, tag="ba1")
    wi2b_s = const.tile([H + 1, 1], F32, tag="wi2b")
    wa2b_s = const.tile([H + 1, E], F32, tag="wa2b")
    gatet_s = const.tile([M, M], F32, tag="gatet")
    rbt_s = const.tile([M, M], F32, tag="rbt")
    for t, d in [
        (rin_s, rin), (wi1t_s, wi1t), (wa1t_s, wa1t),
        (wi2b_s, wi2b), (wa2b_s, wa2b), (gatet_s, gatet), (rbt_s, rbt),
    ]:
        nc.sync.dma_start(out=t[:], in_=d[:])
    nc.sync.dma_start(out=bi1_s[:], in_=bi1v.rearrange("(h one) -> h one", one=1))
    nc.sync.dma_start(out=ba1_s[:], in_=ba1v.rearrange("(h one) -> h one", one=1))

    # router hidden layers, with an extra ones-row to fold the output bias
    hz_ext = small.tile([H + 1, ML], F32, tag="hz")
    ha_ext = small.tile([H + 1, ML], F32, tag="ha")
    for wt, bt, ext in [(wi1t_s, bi1_s, hz_ext), (wa1t_s, ba1_s, ha_ext)]:
        ps = psum.tile([H, ML], F32, tag="rpsum")
        nc.tensor.matmul(ps[:], wt[:], rin_s[:], start=True, stop=True)
        nc.scalar.activation(ext[0:H, :], ps[:], AF.Relu, bias=bt[:])
        nc.vector.memset(ext[H : H + 1, :], 1.0)

    # zeta logits [24,1] -> [8,3] via DRAM bounce
    zps = psum.tile([ML, 1], F32, tag="rpsum")
    nc.tensor.matmul(zps[:], hz_ext[:], wi2b_s[:], start=True, stop=True)
    z24 = small.tile([ML, 1], F32, tag="z24")
    nc.vector.tensor_copy(z24[:], zps[:])
    zdram = dram.tile([ML, 1], F32, tag="zdram")
    nc.sync.dma_start(out=zdram[:], in_=z24[:])
    zl = small.tile([M, L], F32, tag="zl")
    nc.sync.dma_start(out=zl[:], in_=zdram.rearrange("(m l) one -> m (l one)", m=M))

    # alpha logits [24,4]
    aps = psum.tile([ML, E], F32, tag="rpsum")
    nc.tensor.matmul(aps[:], ha_ext[:], wa2b_s[:], start=True, stop=True)
    al = small.tile([ML, E], F32, tag="al")
    nc.vector.tensor_copy(al[:], aps[:])
    if warmup_fn is not None:
        warmup_fn()

    # zeta sparse softmax over L=3, keep top-2 (drop the min)
    zneg = small.tile([M, L], F32, tag="zneg")
    nc.vector.tensor_scalar_mul(zneg[:], zl[:], -1.0)
    zmin = small.tile([M, 1], F32, tag="zmin")
    nc.vector.reduce_max(zmin[:], zneg[:], axis=AX.X)
    nc.vector.tensor_scalar_mul(zmin[:], zmin[:], -1.0)
    zmax = small.tile([M, 1], F32, tag="zmax")
    nc.vector.reduce_max(zmax[:], zl[:], axis=AX.X)
    zmaxn = small.tile([M, 1], F32, tag="zmaxn")
    nc.vector.tensor_scalar_mul(zmaxn[:], zmax[:], -1.0)
    ze = small.tile([M, L], F32, tag="ze")
    nc.scalar.activation(ze[:], zl[:], AF.Exp, bias=zmaxn[:])
    zmask = small.tile([M, L], F32, tag="zmask")
    nc.vector.tensor_scalar(zmask[:], zl[:], zmin[:], None, ALU.is_gt)
    nc.vector.tensor_mul(ze[:], ze[:], zmask[:])
    zs = small.tile([M, 1], F32, tag="zs")
    nc.vector.reduce_sum(zs[:], ze[:], axis=AX.X)
    zrs = small.tile([M, 1], F32, tag="zrs")
    nc.vector.reciprocal(zrs[:], zs[:])
    zeta_all = small.tile([M, L], F32, tag="zeta_all")
    nc.vector.tensor_scalar_mul(zeta_all[:], ze[:], zrs[:])

    # alpha sparse softmax over E=4, keep top-2 (threshold = 2nd max)
    m1 = small.tile([ML, 1], F32, tag="m1")
    nc.vector.reduce_max(m1[:], al[:], axis=AX.X)
    m1n = small.tile([ML, 1], F32, tag="m1n")
    nc.vector.tensor_scalar_mul(m1n[:], m1[:], -1.0)
    meq = small.tile([ML, E], F32, tag="meq")
    nc.vector.tensor_scalar(meq[:], al[:], m1[:], None, ALU.is_equal)
    nc.vector.tensor_scalar_mul(meq[:], meq[:], 1e30)
    v2 = small.tile([ML, E], F32, tag="v2")
    nc.vector.tensor_sub(v2[:], al[:], meq[:])
    m2 = small.tile([ML, 1], F32, tag="m2")
    nc.vector.reduce_max(m2[:], v2[:], axis=AX.X)
    keep = small.tile([ML, E], F32, tag="keep")
    nc.vector.tensor_scalar(keep[:], al[:], m2[:], None, ALU.is_ge)
    ae = small.tile([ML, E], F32, tag="ae")
    nc.scalar.activation(ae[:], al[:], AF.Exp, bias=m1n[:])
    nc.vector.tensor_mul(ae[:], ae[:], keep[:])
    as_ = small.tile([ML, 1], F32, tag="as_")
    nc.vector.reduce_sum(as_[:], ae[:], axis=AX.X)
    ars = small.tile([ML, 1], F32, tag="ars")
    nc.vector.reciprocal(ars[:], as_[:])
    alpha_all = small.tile([ML, E], F32, tag="alpha_all")
    nc.vector.tensor_scalar_mul(alpha_all[:], ae[:], ars[:])

    # [24,4] -> [8,12] via DRAM bounce
    adram = dram.tile([ML, E], F32, tag="adram")
    nc.sync.dma_start(out=adram[:], in_=alpha_all[:])
    alpha8 = small.tile([M, L * E], F32, tag="alpha8")
    nc.sync.dma_start(out=alpha8[:], in_=adram.rearrange("(m l) e -> m (l e)", m=M))

    # RuT[n,m] = softplus(gate[m,n]) * Rb[m,n]   (softplus = ln(1+exp))
    rut = small.tile([M, M], F32, tag="rut")
    nc.scalar.activation(rut[:], gatet_s[:], AF.Exp)
    nc.vector.tensor_scalar_add(rut[:], rut[:], 1.0)
    nc.scalar.activation(rut[:], rut[:], AF.Ln)
    nc.vector.tensor_mul(rut[:], rut[:], rbt_s[:])

    # aggregate [zeta(3) | alpha(12) | ones(1)] through RuT, then normalize
    W16 = L + L * E + 1
    agg_rhs = small.tile([M, W16], F32, tag="agg_rhs")
    nc.vector.tensor_copy(agg_rhs[:, 0:L], zeta_all[:])
    nc.vector.tensor_copy(agg_rhs[:, L : L + L * E], alpha8[:])
    nc.vector.memset(agg_rhs[:, W16 - 1 : W16], 1.0)
    agg_ps = psum.tile([M, W16], F32, tag="rpsum")
    nc.tensor.matmul(agg_ps[:], rut[:], agg_rhs[:], start=True, stop=True)
    rsum = small.tile([M, 1], F32, tag="rsum")
    nc.vector.tensor_scalar_max(rsum[:], agg_ps[:, W16 - 1 : W16], 1e-12)
    rrs = small.tile([M, 1], F32, tag="rrs")
    nc.vector.reciprocal(rrs[:], rsum[:])
    table = small.tile([M, L + L * E], F32, tag="table")
    nc.vector.tensor_scalar_mul(table[:], agg_ps[:, 0 : L + L * E], rrs[:])

    # gamma12[m, l*4+e] = zeta[m,l] * alpha[m, l*4+e]
    zexp = small.tile([M, L * E], F32, tag="zexp")
    zview = zexp.rearrange("p (l e) -> p l e", e=E)
    for e in range(E):
        nc.vector.tensor_copy(zview[:, :, e], table[:, 0:L])
    gamma12 = small.tile([M, L * E], F32, tag="gamma12")
    nc.vector.tensor_mul(gamma12[:], table[:, L : L + L * E], zexp[:])

    # expand over rank r: gexp[:, l*32 + e*8 + r] = gamma12[:, l*4+e]
    gexp = small.tile([M, L * E * RK], F32, tag="gexp")
    gview = gexp.rearrange("p (le r) -> p le r", r=RK)
    for r in range(RK):
        nc.vector.tensor_copy(gview[:, :, r], gamma12[:])
    return gexp


# ---------------------------------------------------------------------------
# Full per-core graph
# ---------------------------------------------------------------------------
def _build(nc):
    DIMS = [(D0, D1), (D1, D2), (D2, D3)]

    xt = nc.dram_tensor("xt", [D0, B_LOC], BF16, kind="ExternalInput")
    onehot = nc.dram_tensor("onehot", [M, B_LOC], F32, kind="ExternalInput")
    combine_d = nc.dram_tensor("combine", [128, 128], BF16, kind="ExternalInput")
    wts = [
        nc.dram_tensor(f"w{l + 1}t", [i, o], BF16, kind="ExternalInput")
        for l, (i, o) in enumerate(DIMS)
    ]
    ats = [
        nc.dram_tensor(f"a{l + 1}t", [i, E * RK], BF16, kind="ExternalInput")
        for l, (i, _) in enumerate(DIMS)
    ]
    lbs = [
        nc.dram_tensor(f"lb{l + 1}", [128, o], BF16, kind="ExternalInput")
        for l, (_, o) in enumerate(DIMS)
    ]
    biases = [
        nc.dram_tensor(f"bias{l + 1}", [o], F32, kind="ExternalInput")
        for l, (_, o) in enumerate(DIMS)
    ]
    out_d = nc.dram_tensor("out", [D3, B_LOC], F32, kind="ExternalOutput")

    with tile.TileContext(nc) as tc:
        with (
            tc.tile_pool(name="const", bufs=1) as const,
            tc.tile_pool(name="small", bufs=1) as small,
            tc.tile_pool(name="rpsum", bufs=1, space="PSUM") as rpsum,
            tc.tile_pool(name="dram", bufs=1, space="DRAM") as dram,
            tc.tile_pool(name="wpool", bufs=1) as wpool,
            tc.tile_pool(name="gpool", bufs=1) as gpool,
            tc.tile_pool(name="onp", bufs=4) as onp,
            tc.tile_pool(name="xcol", bufs=16) as xcolp,
            tc.tile_pool(name="h1", bufs=16) as h1p,
            tc.tile_pool(name="h2", bufs=10) as h2p,
            tc.tile_pool(name="oc", bufs=3) as ocp,
            tc.tile_pool(name="tw", bufs=4) as twp,
            tc.tile_pool(name="mmps", bufs=4, space="PSUM") as mmps,
            tc.tile_pool(name="warmp", bufs=1, space="PSUM") as warmp,
            tc.tile_pool(name="tps", bufs=2, space="PSUM") as tps,
        ):
            # --- PE warmup: keep HAM hot while DMAs stream ------------------
            warm_src = small.tile([128, 128], BF16, tag="warm_src")
            nc.vector.memset(warm_src[:], 0.0)
            warm_sink = dram.tile([128, 128], BF16, tag="warm_sink")
            warm_ps = warmp.tile([128, 128], F32, tag="warm", name="warm_ps")

            def warmup(count, label):
                for i in range(count):
                    nc.tensor.matmul(warm_ps[:], warm_src[:], warm_src[:],
                                     start=True, stop=True)

            # one-hot slices early so their DMAs precede the weight bulk
            on_tiles = []
            for n in range(NT):
                on_t = onp.tile([M, NB], F32, tag="on", name=f"on{n}")
                nc.sync.dma_start(out=on_t[:], in_=onehot[:, n * NB : (n + 1) * NB])
                on_tiles.append(on_t)
            combine_t = const.tile([128, 128], BF16, tag="combine")
            nc.sync.dma_start(out=combine_t[:], in_=combine_d[:])

            warmup(24, "a")
            gexp = _build_routing(nc, const, small, rpsum, dram,
                                  warmup_fn=lambda: warmup(340, "b"))

            # resident weights / inputs: layer-1 + first column first
            w_tiles = [[] for _ in range(L)]
            a_tiles = [[] for _ in range(L)]
            lb_tiles = [None] * L
            b_tiles = [None] * L

            def load_layer_small(l):
                IN, OUT = DIMS[l]
                lb_tiles[l] = wpool.tile([128, OUT], BF16, tag=f"lb{l}", name=f"lb{l}")
                nc.sync.dma_start(out=lb_tiles[l][:], in_=lbs[l][:])
                b_tiles[l] = wpool.tile([128, OUT // 128], F32, tag=f"b{l}", name=f"b{l}")
                nc.sync.dma_start(
                    out=b_tiles[l][:], in_=biases[l].rearrange("(o p) -> p o", p=128)
                )
                for k in range(IN // 128):
                    at_t = wpool.tile([128, E * RK], BF16, tag=f"a{l}_{k}", name=f"a{l}_{k}")
                    nc.sync.dma_start(out=at_t[:], in_=ats[l][k * 128 : (k + 1) * 128, :])
                    a_tiles[l].append(at_t)

            def load_layer(l):
                IN, OUT = DIMS[l]
                nchunk = 4 if l == 0 else 1
                cw = OUT // nchunk
                for k in range(IN // 128):
                    wt_t = wpool.tile([128, OUT], BF16, tag=f"w{l}_{k}", name=f"w{l}_{k}")
                    w_tiles[l].append(wt_t)
                for c in range(nchunk):
                    for k in range(IN // 128):
                        nc.sync.dma_start(
                            out=w_tiles[l][k][:, c * cw : (c + 1) * cw],
                            in_=wts[l][k * 128 : (k + 1) * 128, c * cw : (c + 1) * cw],
                        )

            def load_xcol(n):
                cols = []
                for k in range(D0 // 128):
                    xk = xcolp.tile([128, NB], BF16, tag="xcol", name=f"x{n}_{k}")
                    nc.sync.dma_start(
                        out=xk[:], in_=xt[k * 128 : (k + 1) * 128, n * NB : (n + 1) * NB]
                    )
                    cols.append(xk)
                return cols

            gammas = [
                gpool.tile([128, B_LOC], BF16, tag=f"g{l}", name=f"gamma{l}")
                for l in range(L)
            ]

            def emit_gather():
                for l in range(L):
                    g4 = small.tile([M, 128], F32, tag=f"gexp4_{l}", name=f"gexp4_{l}")
                    for g in range(4):
                        nc.vector.tensor_copy(
                            g4[:, g * 32 : (g + 1) * 32], gexp[:, l * 32 : (l + 1) * 32]
                        )
                    for n in range(NT):
                        gps = tps.tile([128, NB], F32, tag="tpsum")
                        nc.tensor.matmul(gps[:], g4[:], on_tiles[n][:],
                                         start=True, stop=True)
                        nc.vector.tensor_copy(gammas[l][:, n * NB : (n + 1) * NB], gps[:])

            emit_gather()
            warmup(80, "c")
            for l in range(L):
                load_layer_small(l)
            first_cols = load_xcol(0)
            load_layer(0)
            load_layer(1)
            load_layer(2)

            def lora_t4(l, n, cols, KT):
                """LoRA A-side, col-group packed: 4 concurrent partial
                accumulations in one PSUM bank, then one combine matmul that
                also replicates t over the four row groups."""
                part = tps.tile([128, NB], F32, tag="tpsum")
                for k in range(KT):
                    g = k % 4
                    nc.tensor.matmul(
                        part[g * 32 : (g + 1) * 32, :], a_tiles[l][k][:], cols[k][:],
                        start=(k < 4), stop=(k >= KT - 4), tile_position=(0, g * 32),
                    )
                pt = twp.tile([128, NB], BF16, tag="tw", name=f"pt{l}_{n}")
                nc.vector.tensor_copy(pt[:], part[:])
                t4 = tps.tile([128, NB], F32, tag="tpsum")
                nc.tensor.matmul(t4[:], combine_t[:], pt[:], start=True, stop=True)
                return t4

            # main fused pipeline: per batch-column, all three layers
            for n in range(NT):
                cols = first_cols if n == 0 else load_xcol(n)
                for l, (IN, OUT) in enumerate(DIMS):
                    KT, OT = IN // 128, OUT // 128
                    t4 = lora_t4(l, n, cols, KT)
                    tw = twp.tile([128, NB], BF16, tag="tw")
                    nc.vector.tensor_mul(
                        tw[:], t4[:], gammas[l][:, n * NB : (n + 1) * NB]
                    )

                    nxt = []
                    ogroups = list(range(0, OT, 4))
                    for og in ogroups:
                        gw = min(4, OT - og)
                        pss = []
                        for i in range(gw):
                            o = og + i
                            ps = mmps.tile([128, NB], F32, tag="mm")
                            for k in range(KT):
                                nc.tensor.matmul(
                                    ps[:], w_tiles[l][k][:, o * 128 : (o + 1) * 128],
                                    cols[k][:], start=(k == 0), stop=False,
                                )
                            pss.append(ps)
                        for i in range(gw):
                            o = og + i
                            nc.tensor.matmul(
                                pss[i][:],
                                lb_tiles[l][i * 32 : (i + 1) * 32, o * 128 : (o + 1) * 128],
                                tw[i * 32 : (i + 1) * 32, :],
                                start=False, stop=True, tile_position=(i * 32, 0),
                            )
                        for i in range(gw):
                            o = og + i
                            if l < 2:
                                pool = h1p if l == 0 else h2p
                                ot = pool.tile([128, NB], BF16, tag=f"h{l + 1}", name=f"h{l}_{n}_{o}")
                                nc.scalar.activation(
                                    ot[:], pss[i][:], AF.Relu, bias=b_tiles[l][:, o : o + 1]
                                )
                                nxt.append(ot)
                            else:
                                ot = ocp.tile([128, NB], F32, tag="oc", name=f"oc{n}_{o}")
                                nc.scalar.activation(
                                    ot[:], pss[i][:], AF.Relu, bias=b_tiles[l][:, o : o + 1]
                                )
                                nc.sync.dma_start(
                                    out=out_d[o * 128 : (o + 1) * 128, n * NB : (n + 1) * NB],
                                    in_=ot[:],
                                )
                    cols = nxt
            wout = small.tile([128, 128], BF16, tag="warm_out", name="warmout")
            nc.vector.tensor_copy(wout[:], warm_ps[:])
            nc.sync.dma_start(out=warm_sink[:], in_=wout[:])
    return nc


_CACHED = {}


def _get_nc():
    if "nc" not in _CACHED:
        nc = bass.Bass()
        _build(nc)
        _patch_bass_json(nc)
        _CACHED["nc"] = nc
    return _CACHED["nc"]


def kernel(**inputs) -> np.ndarray:
    x = np.asarray(inputs["x"], np.float32)
    ids = np.asarray(inputs["domain_ids"]).astype(np.int64)
    f32 = lambda a: np.ascontiguousarray(np.asarray(a), np.float32)
    bf = lambda a: np.ascontiguousarray(np.asarray(a, np.float32).astype(BF_NP))

    W = [f32(inputs[f"W{i}"]) for i in (1, 2, 3)]
    Bv = [f32(inputs[f"b{i}"]) for i in (1, 2, 3)]
    A = [f32(inputs[f"A{i}"]) for i in (1, 2, 3)]
    Bl = [f32(inputs[f"B{i}"]) for i in (1, 2, 3)]

    dom_emb, layer_pos = f32(inputs["dom_emb"]), f32(inputs["layer_pos"])
    rin = np.concatenate(
        [
            np.broadcast_to(dom_emb[:, None, :], (M, L, H)),
            np.broadcast_to(layer_pos[None, :, :], (M, L, H)),
        ],
        axis=-1,
    ).reshape(M * L, 2 * H).T

    shared = {
        "wi1t": f32(inputs["Wi1"]).T, "wa1t": f32(inputs["Wa1"]).T,
        "bi1v": f32(inputs["bi1"]), "ba1v": f32(inputs["ba1"]),
        "wi2b": np.concatenate([f32(inputs["Wi2"]).T, f32(inputs["bi2"])[None, :]], 0),
        "wa2b": np.concatenate([f32(inputs["Wa2"]).T, f32(inputs["ba2"])[None, :]], 0),
        "gatet": f32(inputs["gate_logits"]).T, "rbt": f32(inputs["R_benefit"]).T,
        "rin": rin,
    }
    shared = {k: f32(v) for k, v in shared.items()}
    shared["combine"] = bf(np.tile(np.eye(E * RK, dtype=np.float32), (4, 4)))
    for l in range(3):
        shared[f"w{l + 1}t"] = bf(W[l].T)
        shared[f"a{l + 1}t"] = bf(A[l].reshape(E * RK, -1).T)
        shared[f"lb{l + 1}"] = bf(np.tile(Bl[l].transpose(0, 2, 1).reshape(E * RK, -1), (4, 1)))
        shared[f"bias{l + 1}"] = Bv[l]

    in_maps = []
    for i in range(N_CORES):
        sl = slice(i * B_LOC, (i + 1) * B_LOC)
        m = dict(shared)
        m["xt"] = bf(x[sl].T)
        m["onehot"] = np.ascontiguousarray(
            (ids[sl][None, :] == np.arange(M)[:, None]).astype(np.float32)
        )
        in_maps.append(m)

    nc = _get_nc()
    res = run_bass_kernel_spmd(nc, in_maps, core_ids=list(range(N_CORES)))
    return np.concatenate(
        [np.asarray(res.results[i]["out"], np.float32).T for i in range(N_CORES)], axis=0
    )
